# revision 5
# baseline (speedup 1.0000x reference)
"""MDTA Trainium2 kernel v2: SBUF-resident pipeline, 8 cores = 4 samples x 2 head-groups."""
import numpy as np
import ml_dtypes
import json as _json
import concourse.bass as bass

# Patch Bass.to_json_bytes: split multi-sem-waits onto same-engine NoOps
# (this walrus build rejects instructions with >1 sync wait).
_orig_tjb = bass.Bass.to_json_bytes
_wctr = [0]

def _split_waits(block):
    out = []
    for ins in block.get("instructions", []):
        si = ins.get("sync_info")
        waits = (si or {}).get("on_wait") or []
        if len(waits) > 1:
            si["on_wait"] = waits[-1:]
            for w in waits[:-1]:
                _wctr[0] += 1
                out.append({"debug": ins.get("debug", 0), "engine": ins["engine"],
                            "ins": [], "outs": [], "name": f"wsplit-{_wctr[0]}",
                            "opcode": "NoOp",
                            "sync_info": {"on_update": [], "on_wait": [w]}})
        out.append(ins)
    block["instructions"] = out
    for sub in block.get("blocks", []):
        _split_waits(sub)

def _patched_tjb(self):
    d = _json.loads(_orig_tjb(self))
    for fn in d.get("functions", []):
        for b in fn.get("blocks", []):
            _split_waits(b)
    return _json.dumps(d).encode()

if not getattr(bass.Bass, "_waitfix_done", False):
    bass.Bass.to_json_bytes = _patched_tjb
    bass.Bass._waitfix_done = True
import concourse.mybir as mybir
from concourse.tile import TileContext
from concourse.bass_utils import run_bass_kernel_spmd

BF = mybir.dt.bfloat16
F32 = mybir.dt.float32
H, W, C = 192, 192, 256
HW = H * W
S = 98  # subband size

DEC_LO = np.array([0.035226291882100656, -0.085441273882241486, -0.13501102001039084,
                   0.45987750211933132, 0.80689150931333875, 0.33267055295095688], dtype=np.float64)
DEC_HI = np.array([-0.33267055295095688, 0.80689150931333875, -0.45987750211933132,
                   -0.13501102001039084, 0.085441273882241486, 0.035226291882100656], dtype=np.float64)
H0A = DEC_LO[::-1].copy()
H1A = DEC_HI[::-1].copy()
G0S = DEC_LO.copy()  # REC_LO reversed = DEC_LO
G1S = np.array([0.035226291882100656, 0.085441273882241486, -0.13501102001039084,
                -0.45987750211933132, 0.80689150931333875, -0.33267055295095688], dtype=np.float64)[::-1].copy()

ALU = mybir.AluOpType
AXL = mybir.AxisListType


def build_core_kernel():
    nc = bass.Bass("TRN2")
    # inputs (per core), x halves pre-cast to bf16 on host
    xa = nc.dram_tensor("xa", [128, H, W], BF, kind="ExternalInput")   # own half (q path + kv cin block 0)
    xb = nc.dram_tensor("xb", [128, H, W], BF, kind="ExternalInput")   # other half (kv cin block 1)
    wkv = nc.dram_tensor("wkv", [2, 128, 256], BF, kind="ExternalInput")   # 1x1 lhsT per cin block
    taps_ab = nc.dram_tensor("taps_ab", [12, 128, 128], BF, kind="ExternalInput")
    taps_de = nc.dram_tensor("taps_de", [12, 128, 128], BF, kind="ExternalInput")
    dwq = nc.dram_tensor("dwq", [128, 36], F32, kind="ExternalInput")   # subband dw per-chan scalars
    dwk = nc.dram_tensor("dwk", [9, 128, 128], BF, kind="ExternalInput")      # k-half dw diag
    dwqd = nc.dram_tensor("dwqd", [18, 128, 128], BF, kind="ExternalInput")   # sb0/sb1 dw diag
    dwv = nc.dram_tensor("dwv", [128, 9], F32, kind="ExternalInput")          # v-half dw per-chan scalars
    projlt = nc.dram_tensor("projlt", [128, 256], BF, kind="ExternalInput")
    tempv = nc.dram_tensor("tempv", [128, 1], F32, kind="ExternalInput")
    identb = nc.dram_tensor("identb", [128, 128], BF, kind="ExternalInput")
    y = nc.dram_tensor("y", [2, 128, HW], BF, kind="ExternalOutput")

    NR1 = 2 * 200 * S      # 39200 loh_pad; also holds zq (4*98*98=38416), qd (36864)
    NR2 = 4 * 100 * 100    # 40000 subb_pad; also holds synth (2*2*96*98=37632), vd (36864)

    with TileContext(nc) as tc:
        with tc.tile_pool(name="const", bufs=1) as cpool, \
             tc.tile_pool(name="work", bufs=3) as pool, \
             tc.tile_pool(name="kvw", bufs=1) as kpool, \
             tc.tile_pool(name="xload", bufs=2) as xpool, \
             tc.tile_pool(name="ypool", bufs=3) as ypool, \
             tc.tile_pool(name="psum", bufs=5, space="PSUM") as pp, \
             tc.tile_pool(name="tpsum", bufs=2, space="PSUM") as tp, \
             tc.tile_pool(name="gsum", bufs=1, space="PSUM") as gp:

            # ---- persistent regions + constants
            R1 = cpool.tile([128, NR1], BF)
            R2 = cpool.tile([128, NR2], BF)
            t_ab = cpool.tile([128, 12, 128], BF)
            nc.sync.dma_start(out=t_ab[:, :, :], in_=taps_ab.rearrange("t p c -> p t c"))
            t_de = t_ab  # reloaded with synthesis taps after stage B
            t_id = cpool.tile([128, 128], BF)
            nc.sync.dma_start(out=t_id[:, :], in_=identb[:, :])
            t_temp = cpool.tile([128, 1], F32)
            nc.sync.dma_start(out=t_temp[:, :], in_=tempv[:, :])

            NBLK = 32  # kv row blocks of 6
            qnp = cpool.tile([128, NBLK], F32)
            knp = cpool.tile([128, NBLK], F32)

            # views
            lohv = R1[:, :2 * 200 * S].rearrange("p (f r j) -> p f r j", f=2, r=200)
            zqv = R1[:, :4 * S * S].rearrange("p (s r j) -> p s r j", s=4, r=S)
            qdf = R1[:, :HW]
            qdp = qdf.rearrange("p (a two w) -> p two a w", two=2, w=W)  # row-parity view
            subv = R2[:, :4 * 100 * 100].rearrange("p (s r j) -> p s r j", s=4, r=100)
            synv = R2[:, :2 * 2 * 96 * S].rearrange("p (f q r j) -> p f q r j", f=2, q=2, r=96)
            vdf = R2[:, :HW]

            # ======== stage A: W-analysis xa -> loh_pad rows 4..196
            nc.vector.memset(lohv[:, :, 0:4, :], 0)
            nc.vector.memset(lohv[:, :, 196:200, :], 0)
            RA = 4
            for r0 in range(0, H, RA):
                rr = min(RA, H - r0)
                xt = xpool.tile([128, RA, 202], BF, tag="xk0")
                nc.vector.memset(xt[:, :, 0:4], 0)
                nc.vector.memset(xt[:, :, 196:202], 0)
                nc.sync.dma_start(out=xt[:, :rr, 4:196], in_=xa[:, r0:r0 + rr, :])
                for f in range(2):
                    ps = pp.tile([128, RA, S], F32, tag="ps")
                    for t in range(6):
                        rhs = xt[:, :rr, t:t + 196].rearrange("p r (j two) -> p two r j", two=2)[:, 0]
                        nc.tensor.matmul(ps[:, :rr, :], t_ab[:, 6 * f + t, :], rhs,
                                         start=(t == 0), stop=(t == 5))
                    nc.scalar.copy(lohv[:, f, 4 + r0:4 + r0 + rr, :], ps[:, :rr, :])

            # deferred constant loads (not needed by stage A)
            t_dwq = cpool.tile([128, 36], F32)
            nc.sync.dma_start(out=t_dwq[:, :], in_=dwq[:, :])
            t_dwqd = cpool.tile([128, 18, 128], BF)
            nc.sync.dma_start(out=t_dwqd[:, :, :], in_=dwqd.rearrange("t p c -> p t c"))
            t_dwvd = cpool.tile([128, 4, 128], BF)
            t_dwk = cpool.tile([128, 9, 128], BF)
            nc.sync.dma_start(out=t_dwk[:, :, :], in_=dwk.rearrange("t p c -> p t c"))
            t_dwv = cpool.tile([128, 9], F32)
            nc.sync.dma_start(out=t_dwv[:, :], in_=dwv[:, :])
            t_wkv = cpool.tile([128, 2, 256], BF)
            nc.sync.dma_start(out=t_wkv[:, :, :], in_=wkv.rearrange("k p c -> p k c"))
            t_proj = cpool.tile([128, 256], BF)
            nc.sync.dma_start(out=t_proj[:, :], in_=projlt[:, :])
            t_temp = cpool.tile([128, 1], F32)
            nc.sync.dma_start(out=t_temp[:, :], in_=tempv[:, :])
            for _i in range(4):
                nc.vector.tensor_scalar_mul(t_dwvd[:, _i, :], t_id[:, :], t_dwv[:, _i:_i + 1])

            # ======== stage B: H-analysis loh -> subb_pad (data at [1:99,1:99])
            nc.vector.memset(subv[:, :, 0, :], 0)
            nc.vector.memset(subv[:, :, 99, :], 0)
            nc.vector.memset(subv[:, :, :, 0:1], 0)
            nc.vector.memset(subv[:, :, :, 99:100], 0)
            lohr = lohv.rearrange("p f (m two) j -> p f two m j", two=2)
            RB = 5
            # B hi-source half (hl, hh) on DVE -- trails stage A's f1 rows
            H0A_T = [0.035226291882100656, -0.085441273882241486, -0.13501102001039084,
                     0.45987750211933132, 0.80689150931333875, 0.33267055295095688][::-1]
            H1A_T = [-0.33267055295095688, 0.80689150931333875, -0.45987750211933132,
                     -0.13501102001039084, 0.085441273882241486, 0.035226291882100656][::-1]
            RBD = 14
            for m0 in range(0, S, RBD):
                mm = min(RBD, S - m0)
                for sb in (2, 3):
                    f_h = sb % 2
                    taps = H0A_T if f_h == 0 else H1A_T
                    dstb = subv[:, sb, 1 + m0:1 + m0 + mm, 1:99]
                    ctmp_f = kpool.tile([128, 1372], BF, tag="tmpv")
                    ctmp = ctmp_f[:, :mm * 98].rearrange("p (r j) -> p r j", j=98)
                    for t in range(6):
                        r = 2 * m0 + t
                        srcv = lohr[:, 1, r % 2, r // 2:r // 2 + mm, :]
                        if t == 0:
                            nc.vector.tensor_scalar_mul(dstb, srcv, float(taps[t]))
                        else:
                            nc.vector.tensor_scalar_mul(ctmp, srcv, float(taps[t]))
                            nc.vector.tensor_tensor(dstb, dstb, ctmp, ALU.add)
            # B lo-source half (ll, lh) on PE
            for m0 in range(0, S, RB):
                mm = min(RB, S - m0)
                for sb in range(2):
                    f_h = sb % 2
                    ps = pp.tile([128, RB, S], F32, tag="ps")
                    for t in range(6):
                        r = 2 * m0 + t
                        rhs = lohr[:, 0, r % 2, r // 2:r // 2 + mm, :]
                        nc.tensor.matmul(ps[:, :mm, :], t_ab[:, 6 * f_h + t, :], rhs,
                                         start=(t == 0), stop=(t == 5))
                    nc.scalar.copy(subv[:, sb, 1 + m0:1 + m0 + mm, 1:99], ps[:, :mm, :])

            # ======== stage C: depthwise 3x3 per subband -> zq (overwrites loh region)
            # plane remap: sb2->plane0, sb3->plane1 (DVE, can trail B); sb0->2, sb1->3 (PE)
            RK = 6
            RC_ = 14
            for m0 in range(0, S, RC_):  # DVE half: sb2, sb3
                for sb in (2, 3):
                    mm = min(RC_, S - m0)
                    zslice = zqv[:, sb - 2, m0:m0 + mm, :]
                    ctmp_f = kpool.tile([128, 1372], BF, tag="tmpv")
                    ctmp = ctmp_f[:, :mm * 98].rearrange("p (r j) -> p r j", j=98)
                    for u in range(3):
                        for v in range(3):
                            t = 3 * u + v
                            srcv = subv[:, sb, m0 + u:m0 + u + mm, v:v + 98]
                            if t == 0:
                                nc.vector.tensor_scalar_mul(zslice, srcv, t_dwq[:, 9 * sb:9 * sb + 1])
                            else:
                                nc.vector.tensor_scalar_mul(ctmp, srcv, t_dwq[:, 9 * sb + t:9 * sb + t + 1])
                                nc.vector.tensor_tensor(zslice, zslice, ctmp, ALU.add)
            RCP = 5
            for sb in (0, 1):  # PE half
                for m0 in range(0, S, RCP):
                    mm = min(RCP, S - m0)
                    ps = pp.tile([128, RCP, S], F32, tag="ps")
                    for u in range(3):
                        for v in range(3):
                            t = 3 * u + v
                            nc.tensor.matmul(ps[:, :mm, :], t_dwqd[:, 9 * sb + t, :],
                                             subv[:, sb, m0 + u:m0 + u + mm, v:v + 98],
                                             start=(t == 0), stop=(t == 8))
                    nc.scalar.copy(zqv[:, sb + 2, m0:m0 + mm, :], ps[:, :mm, :])
            # reload taps tile with synthesis filters (analysis taps dead after B)
            nc.sync.dma_start(out=t_de[:, :, :], in_=taps_de.rearrange("t p c -> p t c"))
            # ======== stage D: H-synthesis zq -> synth (overwrites subb region)
            RD = 5
            for fo in range(2):
                for prr in range(2):
                    for a0 in range(0, 96, RD):
                        aa = min(RD, 96 - a0)
                        ps = pp.tile([128, RD, S], F32, tag="ps")
                        for src in range(2):
                            for d in range(3):
                                ti = 6 * src + (2 * d + 1 - prr)
                                zplane = (2, 3, 0, 1)[2 * fo + src]
                                nc.tensor.matmul(ps[:, :aa, :], t_de[:, ti, :],
                                                 zqv[:, zplane, a0 + d:a0 + d + aa, :],
                                                 start=(src == 0 and d == 0),
                                                 stop=(src == 1 and d == 2))
                        nc.scalar.copy(synv[:, fo, prr, a0:a0 + aa, :], ps[:, :aa, :])

            # ======== stage E: W-synthesis synth -> qd (overwrites zq region)
            RE = 5
            for a0 in range(0, 96, RE):
                aa = min(RE, 96 - a0)
                for prr in range(2):
                    for pc in range(2):
                        ps = pp.tile([128, RE, 96], F32, tag="ps")
                        for src in range(2):
                            for d in range(3):
                                ti = 6 * src + (2 * d + 1 - pc)
                                nc.tensor.matmul(ps[:, :aa, :], t_de[:, ti, :],
                                                 synv[:, src, prr, a0:a0 + aa, d:d + 96],
                                                 start=(src == 0 and d == 0),
                                                 stop=(src == 1 and d == 2))
                        dst = qdp[:, prr, a0:a0 + aa, :].rearrange(
                            "p a (j two) -> p two a j", two=2)[:, pc]
                        nc.scalar.copy(dst, ps[:, :aa, :])

            # q norms on DVE during D/E window (DVE idle there)
            for i in range(NBLK):
                sl_q = qdf[:, 1152 * i:1152 * (i + 1)]
                junkq = kpool.tile([128, 1372], BF, tag="nrm")
                nc.vector.scalar_tensor_tensor(junkq[:, :1152], sl_q, 1.0, sl_q,
                                               ALU.mult, ALU.mult,
                                               accum_out=qnp[:, i:i + 1])
            # ======== phase 2: kv + gram, streaming x in row blocks of 8
            g_ps = gp.tile([128, 128], F32)
            for bi in range(NBLK):
                r0 = bi * RK
                v0 = max(0, r0 - 1)
                v1 = min(H, r0 + RK + 1)
                xt0 = xpool.tile([128, RK + 2, 192], BF, tag="xk0")
                xt1 = xpool.tile([128, RK + 2, 192], BF, tag="xk1")
                for xt, src in ((xt0, xa), (xt1, xb)):
                    nc.sync.dma_start(out=xt[:, v0 - (r0 - 1):v1 - (r0 - 1), :],
                                      in_=src[:, v0:v1, :])
                # 1x1 conv -> k0buf/v0buf (rows r0-1 .. r0+8 in buf rows 0..9)
                k0b = kpool.tile([128, RK + 2, 194], BF, tag="k0b")
                v0b = xpool.tile([128, RK + 2, 194], BF, tag="v0b")
                for dst in (k0b, v0b):
                    nc.vector.memset(dst[:, :, 0:1], 0)
                    nc.vector.memset(dst[:, :, 193:194], 0)
                if r0 == 0:
                    nc.vector.memset(k0b[:, 0, :], 0)
                    nc.vector.memset(v0b[:, 0, :], 0)
                if v1 == H:
                    nc.vector.memset(k0b[:, RK + 1, :], 0)
                    nc.vector.memset(v0b[:, RK + 1, :], 0)
                nrow = v1 - v0
                b0 = v0 - (r0 - 1)
                for g0 in range(0, nrow, 2):
                    gg = min(2, nrow - g0)
                    for mt, dstb in ((0, k0b), (1, v0b)):
                        ps = pp.tile([128, 2, W], F32, tag="ps")
                        for kt, xt in ((0, xt0), (1, xt1)):
                            nc.tensor.matmul(ps[:, :gg, :],
                                             t_wkv[:, kt, 128 * mt:128 * mt + 128],
                                             xt[:, b0 + g0:b0 + g0 + gg, :],
                                             start=(kt == 0), stop=(kt == 1))
                        if mt == 0:
                            nc.scalar.copy(dstb[:, b0 + g0:b0 + g0 + gg, 1:193], ps[:, :gg, :])
                        else:
                            nc.vector.tensor_copy(dstb[:, b0 + g0:b0 + g0 + gg, 1:193], ps[:, :gg, :])
                # dw3x3 k-half on PE -> contiguous ktmp
                ktmp = kpool.tile([128, RK * W], BF, tag="ktmp")
                ktv = ktmp.rearrange("p (r w) -> p r w", w=W)
                for g0 in range(0, RK, 2):
                    ps = pp.tile([128, 2, W], F32, tag="ps")
                    for u in range(3):
                        for v in range(3):
                            t = 3 * u + v
                            nc.tensor.matmul(ps[:, :, :], t_dwk[:, t, :],
                                             k0b[:, g0 + u:g0 + u + 2, v:v + 192],
                                             start=(t == 0), stop=(t == 8))
                    nc.scalar.copy(ktv[:, g0:g0 + 2, :], ps[:, :, :])
                junkk = kpool.tile([128, 1372], BF, tag="nrm")
                nc.vector.scalar_tensor_tensor(junkk[:, :RK * W], ktmp[:, :], 1.0, ktmp[:, :],
                                               ALU.mult, ALU.mult,
                                               accum_out=knp[:, bi:bi + 1])
                # dw3x3 v-half: taps 0,1 on PE -> vd base; taps 2..8 on DVE
                vds = vdf[:, r0 * W:(r0 + RK) * W].rearrange("p (r w) -> p r w", w=W)
                for g0 in range(0, RK, 2):
                    psv = pp.tile([128, 2, W], F32, tag="ps")
                    for t in range(4):
                        nc.tensor.matmul(psv[:, :, :], t_dwvd[:, t, :],
                                         v0b[:, g0 + t // 3:g0 + t // 3 + 2, t % 3:t % 3 + 192],
                                         start=(t == 0), stop=(t == 3))
                    nc.vector.tensor_copy(vds[:, g0:g0 + 2, :], psv[:, :, :])
                tmpv_f = kpool.tile([128, 1372], BF, tag="tmpv")
                tmpv_t = tmpv_f[:, :RK * W].rearrange("p (r w) -> p r w", w=W)
                for u in range(3):
                    for v in range(3):
                        t = 3 * u + v
                        if t < 4:
                            continue
                        srcv = v0b[:, u:u + RK, v:v + 192]
                        nc.vector.tensor_scalar_mul(tmpv_t[:, :, :], srcv, t_dwv[:, t:t + 1])
                        nc.vector.tensor_tensor(vds, vds, tmpv_t[:, :, :], ALU.add)
                # gram: 12 chunks of 128 pixels
                qds = qdf[:, r0 * W:(r0 + RK) * W]
                for ci in range(9):
                    pqt = tp.tile([128, 128], BF, tag="pt")
                    pkt = tp.tile([128, 128], BF, tag="pt")
                    nc.tensor.transpose(pqt[:, :], qds[:, 128 * ci:128 * ci + 128], t_id[:, :])
                    nc.tensor.transpose(pkt[:, :], ktmp[:, 128 * ci:128 * ci + 128], t_id[:, :])
                    qtt = pool.tile([128, 128], BF, tag="qtt")
                    ktt = pool.tile([128, 128], BF, tag="ktt")
                    nc.scalar.copy(qtt[:, :], pqt[:, :])
                    nc.scalar.copy(ktt[:, :], pkt[:, :])
                    nc.tensor.matmul(g_ps[:, :], qtt[:, :], ktt[:, :],
                                     start=(bi == 0 and ci == 0),
                                     stop=(bi == NBLK - 1 and ci == 8))

            # ======== attention block (tiny)
            qn = cpool.tile([128, 1], F32)
            kn = cpool.tile([128, 1], F32)
            nc.vector.tensor_reduce(qn[:, :], qnp[:, :], axis=AXL.X, op=ALU.add)
            nc.vector.tensor_reduce(kn[:, :], knp[:, :], axis=AXL.X, op=ALU.add)
            rq = cpool.tile([128, 1], F32)
            rk = cpool.tile([128, 1], F32)
            nc.vector.tensor_scalar_max(qn[:, :], qn[:, :], 1e-24)
            nc.vector.tensor_scalar_max(kn[:, :], kn[:, :], 1e-24)
            nc.vector.reciprocal(rq[:, :], qn[:, :])
            nc.vector.reciprocal(rk[:, :], kn[:, :])
            nc.scalar.activation(rq[:, :], rq[:, :], mybir.ActivationFunctionType.Sqrt)
            nc.scalar.activation(rk[:, :], rk[:, :], mybir.ActivationFunctionType.Sqrt)
            nc.vector.tensor_mul(rq[:, :], rq[:, :], t_temp[:, :])

            gsb = cpool.tile([128, 128], BF)
            nc.vector.tensor_scalar_mul(gsb[:, :], g_ps[:, :], rq[:, :])
            pt_t = tp.tile([128, 512], BF, tag="pt")
            pt = pt_t[:, :128]
            nc.tensor.transpose(pt[:, :], gsb[:, :], t_id[:, :])
            gtb = cpool.tile([128, 128], BF)
            nc.vector.tensor_scalar_mul(gtb[:, :], pt[:, :], rk[:, :])
            pt2_t = tp.tile([128, 512], BF, tag="pt")
            pt2 = pt2_t[:, :128]
            nc.tensor.transpose(pt2[:, :], gtb[:, :], t_id[:, :])
            gf = cpool.tile([128, 128], F32)
            nc.scalar.copy(gf[:, :], pt2[:, :])

            blk = cpool.tile([128, 32], F32)
            for h in range(4):
                nc.vector.tensor_copy(blk[32 * h:32 * h + 32, :], gf[32 * h:32 * h + 32, 32 * h:32 * h + 32])
            eb = cpool.tile([128, 32], F32)
            nc.scalar.activation(eb[:, :], blk[:, :], mybir.ActivationFunctionType.Exp)
            ssum = cpool.tile([128, 1], F32)
            nc.vector.tensor_reduce(ssum[:, :], eb[:, :], axis=AXL.X, op=ALU.add)
            rs = cpool.tile([128, 1], F32)
            nc.vector.reciprocal(rs[:, :], ssum[:, :])
            nc.vector.tensor_scalar_mul(eb[:, :], eb[:, :], rs[:, :])
            bd = cpool.tile([128, 128], BF)
            nc.vector.memset(bd[:, :], 0)
            for h in range(4):
                nc.vector.tensor_copy(bd[32 * h:32 * h + 32, 32 * h:32 * h + 32], eb[32 * h:32 * h + 32, :])

            mps = pp.tile([128, 256], F32, tag="ps")
            nc.tensor.matmul(mps[:, :], bd[:, :], t_proj[:, :], start=True, stop=True)
            mt_ = cpool.tile([128, 256], BF)
            nc.scalar.copy(mt_[:, :], mps[:, :])

            # ======== y = M @ v (v resident in SBUF)
            for i in range(36):
                for mtile in range(2):
                    ot = ypool.tile([128, 1024], BF, tag="oY")
                    for h in range(2):
                        ps = pp.tile([128, 512], F32, tag="ps")
                        nc.tensor.matmul(ps[:, :], mt_[:, 128 * mtile:128 * mtile + 128],
                                         vdf[:, 1024 * i + 512 * h:1024 * i + 512 * (h + 1)],
                                         start=True, stop=True)
                        nc.scalar.copy(ot[:, 512 * h:512 * h + 300], ps[:, :300])
                        nc.vector.tensor_copy(ot[:, 512 * h + 300:512 * (h + 1)], ps[:, 300:])
                    deng = nc.sync if (i + mtile) % 2 == 0 else nc.gpsimd
                    deng.dma_start(out=y[mtile, :, 1024 * i:1024 * i + 1024], in_=ot[:, :])
    return nc


def _prep_core(x, qkv_w, qkv_conv_w, conv5_w, conv7_w, conv9_w, proj_w, temperature, b, g):
    bf = ml_dtypes.bfloat16
    xb_ = np.asarray(x[b], np.float32)
    sl = slice(128 * g, 128 * g + 128)
    osl = slice(128 * (1 - g), 128 * (1 - g) + 128)
    W_k = qkv_w[sl, :]                                   # (128, 256)
    W_v = qkv_w[256 + 128 * g:256 + 128 * g + 128, :]    # (128, 256)
    wkv = np.zeros((2, 128, 256), np.float32)
    wkv[0] = np.concatenate([W_k[:, sl].T, W_v[:, sl].T], axis=1)
    wkv[1] = np.concatenate([W_k[:, osl].T, W_v[:, osl].T], axis=1)
    taps_ab = np.zeros((12, 128, 128), np.float32)
    taps_de = np.zeros((12, 128, 128), np.float32)
    eye = np.eye(128, dtype=np.float32)
    for t in range(6):
        taps_ab[t] = eye * H0A[t]
        taps_ab[6 + t] = eye * H1A[t]
        taps_de[t] = eye * G0S[t]
        taps_de[6 + t] = eye * G1S[t]
    dwq = np.zeros((128, 36), np.float32)
    dwqd = np.zeros((18, 128, 128), np.float32)
    wq = {0: conv5_w, 1: conv5_w, 2: conv7_w, 3: conv9_w}
    for sb in range(4):
        wloc = wq[sb][sl, 0]
        for t in range(9):
            dwq[:, 9 * sb + t] = wloc[:, t // 3, t % 3]
            if sb < 2:
                dwqd[9 * sb + t] = np.diag(wloc[:, t // 3, t % 3])
    convk = qkv_conv_w[sl, 0]                                      # (128,3,3)
    convv = qkv_conv_w[256 + 128 * g:256 + 128 * g + 128, 0]       # (128,3,3)
    dwk = np.zeros((9, 128, 128), np.float32)
    for t in range(9):
        dwk[t] = np.diag(convk[:, t // 3, t % 3])
    dwv = convv.reshape(128, 9).astype(np.float32)
    projlt = proj_w[:, sl].T.copy()
    tempv = np.repeat(np.asarray(temperature).reshape(8)[4 * g:4 * g + 4], 32).astype(np.float32)[:, None]
    return {
        "xa": xb_[sl].astype(bf), "xb": xb_[osl].astype(bf),
        "wkv": wkv.astype(bf), "taps_ab": taps_ab.astype(bf), "taps_de": taps_de.astype(bf),
        "dwq": dwq, "dwqd": dwqd.astype(bf), "dwk": dwk.astype(bf), "dwv": dwv,
        "projlt": projlt.astype(bf), "tempv": tempv,
        "identb": np.eye(128, dtype=np.float32).astype(bf),
    }


def kernel(x, qkv_w, qkv_conv_w, conv5_w, conv7_w, conv9_w, proj_w, temperature, num_heads):
    x = np.asarray(x, np.float32)
    args = [np.asarray(a, np.float32) for a in
            (qkv_w, qkv_conv_w, conv5_w, conv7_w, conv9_w, proj_w)]
    temperature = np.asarray(temperature, np.float32)
    nc = build_core_kernel()
    in_maps = [_prep_core(x, *args, temperature, core // 2, core % 2) for core in range(8)]
    res = run_bass_kernel_spmd(nc, in_maps, core_ids=list(range(8)))
    out = np.zeros((4, 256, H, W), np.float32)
    for b in range(4):
        acc = res.results[2 * b]["y"].astype(np.float32) + res.results[2 * b + 1]["y"].astype(np.float32)
        out[b] = acc.reshape(256, H, W)
    return out


# revision 6
# speedup vs baseline: 1.0135x; 1.0135x over previous
"""MDTA Trainium2 kernel v2: SBUF-resident pipeline, 8 cores = 4 samples x 2 head-groups."""
import numpy as np
import ml_dtypes
import json as _json
import concourse.bass as bass

# Patch Bass.to_json_bytes: split multi-sem-waits onto same-engine NoOps
# (this walrus build rejects instructions with >1 sync wait).
_orig_tjb = bass.Bass.to_json_bytes
_wctr = [0]

def _split_waits(block):
    out = []
    for ins in block.get("instructions", []):
        si = ins.get("sync_info")
        waits = (si or {}).get("on_wait") or []
        if len(waits) > 1:
            si["on_wait"] = waits[-1:]
            for w in waits[:-1]:
                _wctr[0] += 1
                out.append({"debug": ins.get("debug", 0), "engine": ins["engine"],
                            "ins": [], "outs": [], "name": f"wsplit-{_wctr[0]}",
                            "opcode": "NoOp",
                            "sync_info": {"on_update": [], "on_wait": [w]}})
        out.append(ins)
    block["instructions"] = out
    for sub in block.get("blocks", []):
        _split_waits(sub)

def _patched_tjb(self):
    d = _json.loads(_orig_tjb(self))
    for fn in d.get("functions", []):
        for b in fn.get("blocks", []):
            _split_waits(b)
    return _json.dumps(d).encode()

if not getattr(bass.Bass, "_waitfix_done", False):
    bass.Bass.to_json_bytes = _patched_tjb
    bass.Bass._waitfix_done = True
import concourse.mybir as mybir
from concourse.tile import TileContext
from concourse.bass_utils import run_bass_kernel_spmd

BF = mybir.dt.bfloat16
F32 = mybir.dt.float32
H, W, C = 192, 192, 256
HW = H * W
S = 98  # subband size

DEC_LO = np.array([0.035226291882100656, -0.085441273882241486, -0.13501102001039084,
                   0.45987750211933132, 0.80689150931333875, 0.33267055295095688], dtype=np.float64)
DEC_HI = np.array([-0.33267055295095688, 0.80689150931333875, -0.45987750211933132,
                   -0.13501102001039084, 0.085441273882241486, 0.035226291882100656], dtype=np.float64)
H0A = DEC_LO[::-1].copy()
H1A = DEC_HI[::-1].copy()
G0S = DEC_LO.copy()  # REC_LO reversed = DEC_LO
G1S = np.array([0.035226291882100656, 0.085441273882241486, -0.13501102001039084,
                -0.45987750211933132, 0.80689150931333875, -0.33267055295095688], dtype=np.float64)[::-1].copy()

ALU = mybir.AluOpType
AXL = mybir.AxisListType


def build_core_kernel():
    nc = bass.Bass("TRN2")
    # inputs (per core), x halves pre-cast to bf16 on host
    xa = nc.dram_tensor("xa", [128, H, W], BF, kind="ExternalInput")   # own half (q path + kv cin block 0)
    xb = nc.dram_tensor("xb", [128, H, W], BF, kind="ExternalInput")   # other half (kv cin block 1)
    wkv = nc.dram_tensor("wkv", [2, 128, 256], BF, kind="ExternalInput")   # 1x1 lhsT per cin block
    taps_ab = nc.dram_tensor("taps_ab", [12, 128, 128], BF, kind="ExternalInput")
    taps_de = nc.dram_tensor("taps_de", [12, 128, 128], BF, kind="ExternalInput")
    dwq = nc.dram_tensor("dwq", [128, 36], F32, kind="ExternalInput")   # subband dw per-chan scalars
    dwk = nc.dram_tensor("dwk", [9, 128, 128], BF, kind="ExternalInput")      # k-half dw diag
    dwqd = nc.dram_tensor("dwqd", [18, 128, 128], BF, kind="ExternalInput")   # sb0/sb1 dw diag
    dwv = nc.dram_tensor("dwv", [128, 9], F32, kind="ExternalInput")          # v-half dw per-chan scalars
    projlt = nc.dram_tensor("projlt", [128, 256], BF, kind="ExternalInput")
    tempv = nc.dram_tensor("tempv", [128, 1], F32, kind="ExternalInput")
    identb = nc.dram_tensor("identb", [128, 128], BF, kind="ExternalInput")
    y = nc.dram_tensor("y", [2, 128, HW], BF, kind="ExternalOutput")

    NR1 = 2 * 200 * S      # 39200 loh_pad; also holds zq (4*98*98=38416), qd (36864)
    NR2 = 4 * 100 * 100    # 40000 subb_pad; also holds synth (2*2*96*98=37632), vd (36864)

    with TileContext(nc) as tc:
        with tc.tile_pool(name="const", bufs=1) as cpool, \
             tc.tile_pool(name="work", bufs=3) as pool, \
             tc.tile_pool(name="kvw", bufs=1) as kpool, \
             tc.tile_pool(name="xload", bufs=2) as xpool, \
             tc.tile_pool(name="ypool", bufs=3) as ypool, \
             tc.tile_pool(name="psum", bufs=5, space="PSUM") as pp, \
             tc.tile_pool(name="tpsum", bufs=2, space="PSUM") as tp, \
             tc.tile_pool(name="gsum", bufs=1, space="PSUM") as gp:

            # ---- persistent regions + constants
            R1 = cpool.tile([128, NR1], BF)
            R2 = cpool.tile([128, NR2], BF)
            t_ab = cpool.tile([128, 12, 128], BF)
            nc.sync.dma_start(out=t_ab[:, :, :], in_=taps_ab.rearrange("t p c -> p t c"))
            t_de = t_ab  # reloaded with synthesis taps after stage B
            t_id = cpool.tile([128, 128], BF)
            nc.sync.dma_start(out=t_id[:, :], in_=identb[:, :])
            t_temp = cpool.tile([128, 1], F32)
            nc.sync.dma_start(out=t_temp[:, :], in_=tempv[:, :])

            NBLK = 32  # kv row blocks of 6
            qnp = cpool.tile([128, NBLK], F32)
            knp = cpool.tile([128, NBLK], F32)

            # views
            lohv = R1[:, :2 * 200 * S].rearrange("p (f r j) -> p f r j", f=2, r=200)
            zqv = R1[:, :4 * S * S].rearrange("p (s r j) -> p s r j", s=4, r=S)
            qdf = R1[:, :HW]
            qdp = qdf.rearrange("p (a two w) -> p two a w", two=2, w=W)  # row-parity view
            subv = R2[:, :4 * 100 * 100].rearrange("p (s r j) -> p s r j", s=4, r=100)
            synv = R2[:, :2 * 2 * 96 * S].rearrange("p (f q r j) -> p f q r j", f=2, q=2, r=96)
            vdf = R2[:, :HW]

            # ======== stage A: W-analysis xa -> loh_pad rows 4..196
            nc.vector.memset(lohv[:, :, 0:4, :], 0)
            nc.vector.memset(lohv[:, :, 196:200, :], 0)
            RA = 5
            for r0 in range(0, H, RA):
                rr = min(RA, H - r0)
                xt = xpool.tile([128, RA, 202], BF, tag="xk0")
                nc.vector.memset(xt[:, :, 0:4], 0)
                nc.vector.memset(xt[:, :, 196:202], 0)
                nc.sync.dma_start(out=xt[:, :rr, 4:196], in_=xa[:, r0:r0 + rr, :])
                for f in range(2):
                    ps = pp.tile([128, RA, S], F32, tag="ps")
                    for t in range(6):
                        rhs = xt[:, :rr, t:t + 196].rearrange("p r (j two) -> p two r j", two=2)[:, 0]
                        nc.tensor.matmul(ps[:, :rr, :], t_ab[:, 6 * f + t, :], rhs,
                                         start=(t == 0), stop=(t == 5))
                    nc.scalar.copy(lohv[:, f, 4 + r0:4 + r0 + rr, :], ps[:, :rr, :])

            # deferred constant loads (not needed by stage A)
            t_dwq = cpool.tile([128, 36], F32)
            nc.sync.dma_start(out=t_dwq[:, :], in_=dwq[:, :])
            t_dwqd = cpool.tile([128, 18, 128], BF)
            nc.sync.dma_start(out=t_dwqd[:, :, :], in_=dwqd.rearrange("t p c -> p t c"))
            t_dwvd = cpool.tile([128, 4, 128], BF)
            t_dwk = cpool.tile([128, 9, 128], BF)
            nc.sync.dma_start(out=t_dwk[:, :, :], in_=dwk.rearrange("t p c -> p t c"))
            t_dwv = cpool.tile([128, 9], F32)
            nc.sync.dma_start(out=t_dwv[:, :], in_=dwv[:, :])
            t_wkv = cpool.tile([128, 2, 256], BF)
            nc.sync.dma_start(out=t_wkv[:, :, :], in_=wkv.rearrange("k p c -> p k c"))
            t_proj = cpool.tile([128, 256], BF)
            nc.sync.dma_start(out=t_proj[:, :], in_=projlt[:, :])
            t_temp = cpool.tile([128, 1], F32)
            nc.sync.dma_start(out=t_temp[:, :], in_=tempv[:, :])
            for _i in range(4):
                nc.vector.tensor_scalar_mul(t_dwvd[:, _i, :], t_id[:, :], t_dwv[:, _i:_i + 1])

            # ======== stage B: H-analysis loh -> subb_pad (data at [1:99,1:99])
            nc.vector.memset(subv[:, :, 0, :], 0)
            nc.vector.memset(subv[:, :, 99, :], 0)
            nc.vector.memset(subv[:, :, :, 0:1], 0)
            nc.vector.memset(subv[:, :, :, 99:100], 0)
            lohr = lohv.rearrange("p f (m two) j -> p f two m j", two=2)
            RB = 5
            # B hi-source half (hl, hh) on DVE -- trails stage A's f1 rows
            H0A_T = [0.035226291882100656, -0.085441273882241486, -0.13501102001039084,
                     0.45987750211933132, 0.80689150931333875, 0.33267055295095688][::-1]
            H1A_T = [-0.33267055295095688, 0.80689150931333875, -0.45987750211933132,
                     -0.13501102001039084, 0.085441273882241486, 0.035226291882100656][::-1]
            RBD = 14
            for m0 in range(0, S, RBD):
                mm = min(RBD, S - m0)
                for sb in (2, 3):
                    f_h = sb % 2
                    taps = H0A_T if f_h == 0 else H1A_T
                    dstb = subv[:, sb, 1 + m0:1 + m0 + mm, 1:99]
                    ctmp_f = kpool.tile([128, 1372], BF, tag="tmpv")
                    ctmp = ctmp_f[:, :mm * 98].rearrange("p (r j) -> p r j", j=98)
                    for t in range(6):
                        r = 2 * m0 + t
                        srcv = lohr[:, 1, r % 2, r // 2:r // 2 + mm, :]
                        if t == 0:
                            nc.vector.tensor_scalar_mul(dstb, srcv, float(taps[t]))
                        else:
                            nc.vector.tensor_scalar_mul(ctmp, srcv, float(taps[t]))
                            nc.vector.tensor_tensor(dstb, dstb, ctmp, ALU.add)
            # B lo-source half (ll, lh) on PE
            for m0 in range(0, S, RB):
                mm = min(RB, S - m0)
                for sb in range(2):
                    f_h = sb % 2
                    ps = pp.tile([128, RB, S], F32, tag="ps")
                    for t in range(6):
                        r = 2 * m0 + t
                        rhs = lohr[:, 0, r % 2, r // 2:r // 2 + mm, :]
                        nc.tensor.matmul(ps[:, :mm, :], t_ab[:, 6 * f_h + t, :], rhs,
                                         start=(t == 0), stop=(t == 5))
                    nc.scalar.copy(subv[:, sb, 1 + m0:1 + m0 + mm, 1:99], ps[:, :mm, :])

            # ======== stage C: depthwise 3x3 per subband -> zq (overwrites loh region)
            # plane remap: sb2->plane0, sb3->plane1 (DVE, can trail B); sb0->2, sb1->3 (PE)
            RK = 6
            RC_ = 14
            for m0 in range(0, S, RC_):  # DVE half: sb2, sb3
                for sb in (2, 3):
                    mm = min(RC_, S - m0)
                    zslice = zqv[:, sb - 2, m0:m0 + mm, :]
                    ctmp_f = kpool.tile([128, 1372], BF, tag="tmpv")
                    ctmp = ctmp_f[:, :mm * 98].rearrange("p (r j) -> p r j", j=98)
                    for u in range(3):
                        for v in range(3):
                            t = 3 * u + v
                            srcv = subv[:, sb, m0 + u:m0 + u + mm, v:v + 98]
                            if t == 0:
                                nc.vector.tensor_scalar_mul(zslice, srcv, t_dwq[:, 9 * sb:9 * sb + 1])
                            else:
                                nc.vector.tensor_scalar_mul(ctmp, srcv, t_dwq[:, 9 * sb + t:9 * sb + t + 1])
                                nc.vector.tensor_tensor(zslice, zslice, ctmp, ALU.add)
            RCP = 5
            for sb in (0, 1):  # PE half
                for m0 in range(0, S, RCP):
                    mm = min(RCP, S - m0)
                    ps = pp.tile([128, RCP, S], F32, tag="ps")
                    for u in range(3):
                        for v in range(3):
                            t = 3 * u + v
                            nc.tensor.matmul(ps[:, :mm, :], t_dwqd[:, 9 * sb + t, :],
                                             subv[:, sb, m0 + u:m0 + u + mm, v:v + 98],
                                             start=(t == 0), stop=(t == 8))
                    nc.scalar.copy(zqv[:, sb + 2, m0:m0 + mm, :], ps[:, :mm, :])
            # reload taps tile with synthesis filters (analysis taps dead after B)
            nc.sync.dma_start(out=t_de[:, :, :], in_=taps_de.rearrange("t p c -> p t c"))
            # ======== stage D: H-synthesis zq -> synth (overwrites subb region)
            RD = 5
            for fo in range(2):
                for prr in range(2):
                    for a0 in range(0, 96, RD):
                        aa = min(RD, 96 - a0)
                        ps = pp.tile([128, RD, S], F32, tag="ps")
                        for src in range(2):
                            for d in range(3):
                                ti = 6 * src + (2 * d + 1 - prr)
                                zplane = (2, 3, 0, 1)[2 * fo + src]
                                nc.tensor.matmul(ps[:, :aa, :], t_de[:, ti, :],
                                                 zqv[:, zplane, a0 + d:a0 + d + aa, :],
                                                 start=(src == 0 and d == 0),
                                                 stop=(src == 1 and d == 2))
                        nc.scalar.copy(synv[:, fo, prr, a0:a0 + aa, :], ps[:, :aa, :])

            # ======== stage E: W-synthesis synth -> qd (overwrites zq region)
            RE = 5
            for a0 in range(0, 96, RE):
                aa = min(RE, 96 - a0)
                for prr in range(2):
                    for pc in range(2):
                        ps = pp.tile([128, RE, 96], F32, tag="ps")
                        for src in range(2):
                            for d in range(3):
                                ti = 6 * src + (2 * d + 1 - pc)
                                nc.tensor.matmul(ps[:, :aa, :], t_de[:, ti, :],
                                                 synv[:, src, prr, a0:a0 + aa, d:d + 96],
                                                 start=(src == 0 and d == 0),
                                                 stop=(src == 1 and d == 2))
                        dst = qdp[:, prr, a0:a0 + aa, :].rearrange(
                            "p a (j two) -> p two a j", two=2)[:, pc]
                        nc.scalar.copy(dst, ps[:, :aa, :])

            # q norms on DVE during D/E window (DVE idle there)
            for i in range(NBLK):
                sl_q = qdf[:, 1152 * i:1152 * (i + 1)]
                junkq = kpool.tile([128, 1372], BF, tag="nrm")
                nc.vector.scalar_tensor_tensor(junkq[:, :1152], sl_q, 1.0, sl_q,
                                               ALU.mult, ALU.mult,
                                               accum_out=qnp[:, i:i + 1])
            # ======== phase 2: kv + gram, streaming x in row blocks of 8
            g_ps = gp.tile([128, 128], F32)
            for bi in range(NBLK):
                r0 = bi * RK
                v0 = max(0, r0 - 1)
                v1 = min(H, r0 + RK + 1)
                xt0 = xpool.tile([128, RK + 2, 192], BF, tag="xk0")
                xt1 = xpool.tile([128, RK + 2, 192], BF, tag="xk1")
                for xt, src in ((xt0, xa), (xt1, xb)):
                    nc.sync.dma_start(out=xt[:, v0 - (r0 - 1):v1 - (r0 - 1), :],
                                      in_=src[:, v0:v1, :])
                # 1x1 conv -> k0buf/v0buf (rows r0-1 .. r0+8 in buf rows 0..9)
                k0b = kpool.tile([128, RK + 2, 194], BF, tag="k0b")
                v0b = xpool.tile([128, RK + 2, 194], BF, tag="v0b")
                for dst in (k0b, v0b):
                    nc.vector.memset(dst[:, :, 0:1], 0)
                    nc.vector.memset(dst[:, :, 193:194], 0)
                if r0 == 0:
                    nc.vector.memset(k0b[:, 0, :], 0)
                    nc.vector.memset(v0b[:, 0, :], 0)
                if v1 == H:
                    nc.vector.memset(k0b[:, RK + 1, :], 0)
                    nc.vector.memset(v0b[:, RK + 1, :], 0)
                nrow = v1 - v0
                b0 = v0 - (r0 - 1)
                for g0 in range(0, nrow, 2):
                    gg = min(2, nrow - g0)
                    for mt, dstb in ((0, k0b), (1, v0b)):
                        ps = pp.tile([128, 2, W], F32, tag="ps")
                        for kt, xt in ((0, xt0), (1, xt1)):
                            nc.tensor.matmul(ps[:, :gg, :],
                                             t_wkv[:, kt, 128 * mt:128 * mt + 128],
                                             xt[:, b0 + g0:b0 + g0 + gg, :],
                                             start=(kt == 0), stop=(kt == 1))
                        if mt == 0:
                            nc.scalar.copy(dstb[:, b0 + g0:b0 + g0 + gg, 1:193], ps[:, :gg, :])
                        else:
                            nc.vector.tensor_copy(dstb[:, b0 + g0:b0 + g0 + gg, 1:193], ps[:, :gg, :])
                # dw3x3 k-half on PE -> contiguous ktmp
                ktmp = kpool.tile([128, RK * W], BF, tag="ktmp")
                ktv = ktmp.rearrange("p (r w) -> p r w", w=W)
                for g0 in range(0, RK, 2):
                    ps = pp.tile([128, 2, W], F32, tag="ps")
                    for u in range(3):
                        for v in range(3):
                            t = 3 * u + v
                            nc.tensor.matmul(ps[:, :, :], t_dwk[:, t, :],
                                             k0b[:, g0 + u:g0 + u + 2, v:v + 192],
                                             start=(t == 0), stop=(t == 8))
                    nc.scalar.copy(ktv[:, g0:g0 + 2, :], ps[:, :, :])
                junkk = kpool.tile([128, 1372], BF, tag="nrm")
                nc.vector.scalar_tensor_tensor(junkk[:, :RK * W], ktmp[:, :], 1.0, ktmp[:, :],
                                               ALU.mult, ALU.mult,
                                               accum_out=knp[:, bi:bi + 1])
                # dw3x3 v-half: taps 0,1 on PE -> vd base; taps 2..8 on DVE
                vds = vdf[:, r0 * W:(r0 + RK) * W].rearrange("p (r w) -> p r w", w=W)
                for g0 in range(0, RK, 2):
                    psv = pp.tile([128, 2, W], F32, tag="ps")
                    for t in range(4):
                        nc.tensor.matmul(psv[:, :, :], t_dwvd[:, t, :],
                                         v0b[:, g0 + t // 3:g0 + t // 3 + 2, t % 3:t % 3 + 192],
                                         start=(t == 0), stop=(t == 3))
                    nc.vector.tensor_copy(vds[:, g0:g0 + 2, :], psv[:, :, :])
                tmpv_f = kpool.tile([128, 1372], BF, tag="tmpv")
                tmpv_t = tmpv_f[:, :RK * W].rearrange("p (r w) -> p r w", w=W)
                for u in range(3):
                    for v in range(3):
                        t = 3 * u + v
                        if t < 4:
                            continue
                        srcv = v0b[:, u:u + RK, v:v + 192]
                        nc.vector.tensor_scalar_mul(tmpv_t[:, :, :], srcv, t_dwv[:, t:t + 1])
                        nc.vector.tensor_tensor(vds, vds, tmpv_t[:, :, :], ALU.add)
                # gram: 9 chunks; transposes packed 4-per-bank, gram mms interleaved
                qds = qdf[:, r0 * W:(r0 + RK) * W]
                qksb = []
                for ti in range(5):
                    n_sl = min(4, 18 - 4 * ti)
                    p4 = tp.tile([128, 512], BF, tag="pt")
                    q4 = pool.tile([128, 512], BF, tag="qk")
                    for si in range(n_sl):
                        gidx = 4 * ti + si
                        ci = gidx // 2
                        src = qds if gidx % 2 == 0 else ktmp
                        nc.tensor.transpose(p4[:, 128 * si:128 * si + 128],
                                            src[:, 128 * ci:128 * ci + 128], t_id[:, :])
                    nc.scalar.copy(q4[:, :128 * n_sl], p4[:, :128 * n_sl])
                    qksb.append(q4)
                    for ci in range(9):
                        if (2 * ci + 1) // 4 != ti:
                            continue
                        qt_t = qksb[(2 * ci) // 4]
                        kt_t = qksb[(2 * ci + 1) // 4]
                        nc.tensor.matmul(g_ps[:, :],
                                         qt_t[:, 128 * ((2 * ci) % 4):128 * ((2 * ci) % 4) + 128],
                                         kt_t[:, 128 * ((2 * ci + 1) % 4):128 * ((2 * ci + 1) % 4) + 128],
                                         start=(bi == 0 and ci == 0),
                                         stop=(bi == NBLK - 1 and ci == 8))

            # ======== attention block (tiny)
            qn = cpool.tile([128, 1], F32)
            kn = cpool.tile([128, 1], F32)
            nc.vector.tensor_reduce(qn[:, :], qnp[:, :], axis=AXL.X, op=ALU.add)
            nc.vector.tensor_reduce(kn[:, :], knp[:, :], axis=AXL.X, op=ALU.add)
            rq = cpool.tile([128, 1], F32)
            rk = cpool.tile([128, 1], F32)
            nc.vector.tensor_scalar_max(qn[:, :], qn[:, :], 1e-24)
            nc.vector.tensor_scalar_max(kn[:, :], kn[:, :], 1e-24)
            nc.vector.reciprocal(rq[:, :], qn[:, :])
            nc.vector.reciprocal(rk[:, :], kn[:, :])
            nc.scalar.activation(rq[:, :], rq[:, :], mybir.ActivationFunctionType.Sqrt)
            nc.scalar.activation(rk[:, :], rk[:, :], mybir.ActivationFunctionType.Sqrt)
            nc.vector.tensor_mul(rq[:, :], rq[:, :], t_temp[:, :])

            gsb = cpool.tile([128, 128], BF)
            nc.vector.tensor_scalar_mul(gsb[:, :], g_ps[:, :], rq[:, :])
            pt_t = tp.tile([128, 512], BF, tag="pt")
            pt = pt_t[:, :128]
            nc.tensor.transpose(pt[:, :], gsb[:, :], t_id[:, :])
            gtb = cpool.tile([128, 128], BF)
            nc.vector.tensor_scalar_mul(gtb[:, :], pt[:, :], rk[:, :])
            pt2_t = tp.tile([128, 512], BF, tag="pt")
            pt2 = pt2_t[:, :128]
            nc.tensor.transpose(pt2[:, :], gtb[:, :], t_id[:, :])
            gf = cpool.tile([128, 128], F32)
            nc.scalar.copy(gf[:, :], pt2[:, :])

            blk = cpool.tile([128, 32], F32)
            for h in range(4):
                nc.vector.tensor_copy(blk[32 * h:32 * h + 32, :], gf[32 * h:32 * h + 32, 32 * h:32 * h + 32])
            eb = cpool.tile([128, 32], F32)
            nc.scalar.activation(eb[:, :], blk[:, :], mybir.ActivationFunctionType.Exp)
            ssum = cpool.tile([128, 1], F32)
            nc.vector.tensor_reduce(ssum[:, :], eb[:, :], axis=AXL.X, op=ALU.add)
            rs = cpool.tile([128, 1], F32)
            nc.vector.reciprocal(rs[:, :], ssum[:, :])
            nc.vector.tensor_scalar_mul(eb[:, :], eb[:, :], rs[:, :])
            bd = cpool.tile([128, 128], BF)
            nc.vector.memset(bd[:, :], 0)
            for h in range(4):
                nc.vector.tensor_copy(bd[32 * h:32 * h + 32, 32 * h:32 * h + 32], eb[32 * h:32 * h + 32, :])

            mps = pp.tile([128, 256], F32, tag="ps")
            nc.tensor.matmul(mps[:, :], bd[:, :], t_proj[:, :], start=True, stop=True)
            mt_ = cpool.tile([128, 256], BF)
            nc.scalar.copy(mt_[:, :], mps[:, :])

            # ======== y = M @ v (v resident in SBUF)
            for i in range(36):
                for mtile in range(2):
                    ot = ypool.tile([128, 1024], BF, tag="oY")
                    for h in range(2):
                        ps = pp.tile([128, 512], F32, tag="ps")
                        nc.tensor.matmul(ps[:, :], mt_[:, 128 * mtile:128 * mtile + 128],
                                         vdf[:, 1024 * i + 512 * h:1024 * i + 512 * (h + 1)],
                                         start=True, stop=True)
                        nc.scalar.copy(ot[:, 512 * h:512 * h + 300], ps[:, :300])
                        nc.vector.tensor_copy(ot[:, 512 * h + 300:512 * (h + 1)], ps[:, 300:])
                    deng = nc.sync if (i + mtile) % 2 == 0 else nc.gpsimd
                    deng.dma_start(out=y[mtile, :, 1024 * i:1024 * i + 1024], in_=ot[:, :])
    return nc


def _prep_core(x, qkv_w, qkv_conv_w, conv5_w, conv7_w, conv9_w, proj_w, temperature, b, g):
    bf = ml_dtypes.bfloat16
    xb_ = np.asarray(x[b], np.float32)
    sl = slice(128 * g, 128 * g + 128)
    osl = slice(128 * (1 - g), 128 * (1 - g) + 128)
    W_k = qkv_w[sl, :]                                   # (128, 256)
    W_v = qkv_w[256 + 128 * g:256 + 128 * g + 128, :]    # (128, 256)
    wkv = np.zeros((2, 128, 256), np.float32)
    wkv[0] = np.concatenate([W_k[:, sl].T, W_v[:, sl].T], axis=1)
    wkv[1] = np.concatenate([W_k[:, osl].T, W_v[:, osl].T], axis=1)
    taps_ab = np.zeros((12, 128, 128), np.float32)
    taps_de = np.zeros((12, 128, 128), np.float32)
    eye = np.eye(128, dtype=np.float32)
    for t in range(6):
        taps_ab[t] = eye * H0A[t]
        taps_ab[6 + t] = eye * H1A[t]
        taps_de[t] = eye * G0S[t]
        taps_de[6 + t] = eye * G1S[t]
    dwq = np.zeros((128, 36), np.float32)
    dwqd = np.zeros((18, 128, 128), np.float32)
    wq = {0: conv5_w, 1: conv5_w, 2: conv7_w, 3: conv9_w}
    for sb in range(4):
        wloc = wq[sb][sl, 0]
        for t in range(9):
            dwq[:, 9 * sb + t] = wloc[:, t // 3, t % 3]
            if sb < 2:
                dwqd[9 * sb + t] = np.diag(wloc[:, t // 3, t % 3])
    convk = qkv_conv_w[sl, 0]                                      # (128,3,3)
    convv = qkv_conv_w[256 + 128 * g:256 + 128 * g + 128, 0]       # (128,3,3)
    dwk = np.zeros((9, 128, 128), np.float32)
    for t in range(9):
        dwk[t] = np.diag(convk[:, t // 3, t % 3])
    dwv = convv.reshape(128, 9).astype(np.float32)
    projlt = proj_w[:, sl].T.copy()
    tempv = np.repeat(np.asarray(temperature).reshape(8)[4 * g:4 * g + 4], 32).astype(np.float32)[:, None]
    return {
        "xa": xb_[sl].astype(bf), "xb": xb_[osl].astype(bf),
        "wkv": wkv.astype(bf), "taps_ab": taps_ab.astype(bf), "taps_de": taps_de.astype(bf),
        "dwq": dwq, "dwqd": dwqd.astype(bf), "dwk": dwk.astype(bf), "dwv": dwv,
        "projlt": projlt.astype(bf), "tempv": tempv,
        "identb": np.eye(128, dtype=np.float32).astype(bf),
    }


def kernel(x, qkv_w, qkv_conv_w, conv5_w, conv7_w, conv9_w, proj_w, temperature, num_heads):
    x = np.asarray(x, np.float32)
    args = [np.asarray(a, np.float32) for a in
            (qkv_w, qkv_conv_w, conv5_w, conv7_w, conv9_w, proj_w)]
    temperature = np.asarray(temperature, np.float32)
    nc = build_core_kernel()
    in_maps = [_prep_core(x, *args, temperature, core // 2, core % 2) for core in range(8)]
    res = run_bass_kernel_spmd(nc, in_maps, core_ids=list(range(8)))
    out = np.zeros((4, 256, H, W), np.float32)
    for b in range(4):
        acc = res.results[2 * b]["y"].astype(np.float32) + res.results[2 * b + 1]["y"].astype(np.float32)
        out[b] = acc.reshape(256, H, W)
    return out
